# revision 8
# baseline (speedup 1.0000x reference)
"""DistMult edge scoring on 8 Trainium2 NeuronCores.

score[e] = sum_d h[src[e],d] * fwd_rel[etype[e],d] * h[dst[e],d]

Strategy (edge-parallel): shard the 640k edges across 8 cores (80k each).
Each core gathers h[src], h[dst] (one merged dma_gather per tile: both
index the node table) and fwd_rel[etype] rows from HBM via the dma_gather
custom instruction (SWDGE), multiplies elementwise and reduces along D.

Variants:
  f32  - fp32 gathers, edges-on-partitions ([128, T/128, 128] tiles),
         2 DVE muls + DVE reduce along the free axis.
  f16e - tables downcast to fp16 on device first (halves gather traffic);
         same dataflow as f32, DVE muls at 2x perf mode.
  f16d - fp16 with transposed gathers (D-on-partitions, [128, T] tiles);
         muls on DVE at 2x, reduction over D on TensorE as a
         [128,128]x[128,1] matmul with a ones vector per edge block.

Index tensors are pre-marshalled on host into dma_gather's required wrap:
int16, index i at (partition i%16, column i//16), replicated 8x across the
128 partitions (one copy per GpSimd Q7 core). The src/dst streams are
concatenated per tile (src block then dst block) for the merged gather.

dma_gather calls use single_packet=False: single-packet mode hangs the
device above ~768 indices per call (HW-probed).
"""

import os
import sys

import numpy as np

sys.path.insert(0, "/opt/trn_rl_repo")

import concourse.bass as bass
import concourse.mybir as mybir
from concourse import bacc
from concourse.tile import TileContext

N_NODES = 10000
N_EDGES = 640000
D = 128
NUM_RELS = 500
N_CORES = 8
CORE_E = N_EDGES // N_CORES  # 80000

F32 = mybir.dt.float32
F16 = mybir.dt.float16
I16 = mybir.dt.int16


def _declare_io(nc, nt, t16, tcols):
    h = nc.declare_dram_parameter("h", [N_NODES, D], F32, isOutput=False)
    rel = nc.declare_dram_parameter("fwd_rel", [NUM_RELS, D], F32, isOutput=False)
    isd = nc.declare_dram_parameter("idx_sd", [nt, 128, 2 * t16], I16, isOutput=False)
    iet = nc.declare_dram_parameter("idx_et", [nt, 128, t16], I16, isOutput=False)
    out = nc.declare_dram_parameter("scores", [nt, 128, tcols], F32, isOutput=True)
    return h, rel, isd, iet, out


def _cast_table_f16(nc, tc, src_dram, dst_dram, n_rows):
    """Downcast a [n_rows, D] fp32 DRAM table to fp16 via an SBUF bounce.

    The flat element stream is reinterpreted as [128, n_rows*D/128]; the
    cast is elementwise so row alignment is irrelevant.
    """
    total = n_rows * D
    free = total // 128
    with tc.tile_pool(name=f"cast_{dst_dram.name}", bufs=1) as pool:
        a = pool.tile([128, free], F32)
        b = pool.tile([128, free], F16)
        src_flat = src_dram[:].rearrange("a d -> (a d)").rearrange("(p f) -> p f", p=128)
        dst_flat = dst_dram[:].rearrange("a d -> (a d)").rearrange("(p f) -> p f", p=128)
        nc.sync.dma_start(out=a[:], in_=src_flat)
        nc.vector.tensor_copy(b[:], a[:])
        nc.sync.dma_start(out=dst_flat, in_=b[:])


def build_program(core_e: int, tile_t: int, variant: str = "f32"):
    """Build the per-core Bass program (SPMD: same program, 8 cores)."""
    assert core_e % tile_t == 0 and tile_t % 128 == 0
    nt = core_e // tile_t  # tiles per core
    t16 = tile_t // 16  # idx wrap columns
    tcols = tile_t // 128  # gathered columns per partition

    nc = bacc.Bacc()
    h, rel, isd, iet, out = _declare_io(nc, nt, t16, tcols)

    with TileContext(nc) as tc:
        if variant == "f32":
            _body_e(nc, tc, h, rel, isd, iet, out, nt, t16, tcols, tile_t, F32)
        elif variant == "f16e":
            h16 = nc.dram_tensor("h16", [N_NODES, D], F16)
            rel16 = nc.dram_tensor("rel16", [NUM_RELS, D], F16)
            _cast_table_f16(nc, tc, h, h16, N_NODES)
            _cast_table_f16(nc, tc, rel, rel16, NUM_RELS)
            _body_e(nc, tc, h16, rel16, isd, iet, out, nt, t16, tcols, tile_t, F16)
        elif variant == "f16d":
            h16 = nc.dram_tensor("h16", [N_NODES, D], F16)
            rel16 = nc.dram_tensor("rel16", [NUM_RELS, D], F16)
            _cast_table_f16(nc, tc, h, h16, N_NODES)
            _cast_table_f16(nc, tc, rel, rel16, NUM_RELS)
            _body_f16d(nc, tc, h16, rel16, isd, iet, out, nt, t16, tcols, tile_t)
        else:
            raise ValueError(variant)

    nc.compile()
    return nc


def _body_e(nc, tc, htab, wtab, isd, iet, out, nt, t16, tcols, tile_t, dt):
    """Edges-on-partitions dataflow (f32 or f16 tables)."""
    with (
        tc.tile_pool(name="gat", bufs=2) as gp,
        tc.tile_pool(name="idx", bufs=2) as ip,
        tc.tile_pool(name="res", bufs=2) as rp,
    ):
        for t in range(nt):
            ix_sd = ip.tile([128, 2 * t16], I16, tag="ixsd")
            ix_w = ip.tile([128, t16], I16, tag="ixw")
            nc.sync.dma_start(out=ix_sd[:], in_=isd[t])
            nc.sync.dma_start(out=ix_w[:], in_=iet[t])

            uv = gp.tile([128, 2 * tcols, D], dt, tag="uv")
            w = gp.tile([128, tcols, D], dt, tag="w")
            nc.gpsimd.dma_gather(
                out_ap=uv[:], in_ap=htab[:], idxs_ap=ix_sd[:],
                num_idxs=2 * tile_t, num_idxs_reg=2 * tile_t, elem_size=D,
                single_packet=False,
            )
            nc.gpsimd.dma_gather(
                out_ap=w[:], in_ap=wtab[:], idxs_ap=ix_w[:],
                num_idxs=tile_t, num_idxs_reg=tile_t, elem_size=D,
                single_packet=False,
            )

            p = gp.tile([128, tcols, D], dt, tag="p")
            nc.vector.tensor_mul(p[:], uv[:, :tcols, :], w[:])
            nc.vector.tensor_mul(p[:], p[:], uv[:, tcols:, :])
            s = rp.tile([128, tcols], F32, tag="s")
            nc.vector.reduce_sum(s[:], p[:], axis=mybir.AxisListType.X)
            nc.sync.dma_start(out=out[t], in_=s[:])


def _body_f16d(nc, tc, h16, rel16, isd, iet, out, nt, t16, tcols, tile_t):
    """D-on-partitions dataflow: transposed fp16 gathers, PE reduction."""
    with (
        tc.tile_pool(name="ones", bufs=1) as onep,
        tc.tile_pool(name="gat", bufs=2) as gp,
        tc.tile_pool(name="idx", bufs=2) as ip,
        tc.tile_pool(name="ps", bufs=2, space="PSUM") as pp,
        tc.tile_pool(name="res", bufs=2) as rp,
    ):
        ones = onep.tile([128, 1], F16)
        nc.gpsimd.memset(ones[:], 1.0)

        for t in range(nt):
            ix_sd = ip.tile([128, 2 * t16], I16, tag="ixsd")
            ix_w = ip.tile([128, t16], I16, tag="ixw")
            nc.sync.dma_start(out=ix_sd[:], in_=isd[t])
            nc.sync.dma_start(out=ix_w[:], in_=iet[t])

            # transposed gathers: D on partitions, edges on the free axis
            uv = gp.tile([128, 1, 2 * tile_t], F16, tag="uv")
            w = gp.tile([128, 1, tile_t], F16, tag="w")
            nc.gpsimd.dma_gather(
                out_ap=uv[:], in_ap=h16[:], idxs_ap=ix_sd[:],
                num_idxs=2 * tile_t, num_idxs_reg=2 * tile_t, elem_size=D,
                transpose=True, single_packet=False,
            )
            nc.gpsimd.dma_gather(
                out_ap=w[:], in_ap=rel16[:], idxs_ap=ix_w[:],
                num_idxs=tile_t, num_idxs_reg=tile_t, elem_size=D,
                transpose=True, single_packet=False,
            )

            p = gp.tile([128, 1, tile_t], F16, tag="p")
            nc.vector.tensor_mul(p[:], uv[:, :, :tile_t], w[:])
            nc.vector.tensor_mul(p[:], p[:], uv[:, :, tile_t:])

            ps = pp.tile([128, tcols], F32, tag="ps")
            for c in range(tcols):
                nc.tensor.matmul(
                    out=ps[:, c : c + 1],
                    lhsT=p[:, 0, c * 128 : (c + 1) * 128],
                    rhs=ones[:],
                    start=True,
                    stop=True,
                )
            s = rp.tile([128, tcols], F32, tag="s")
            nc.scalar.copy(s[:], ps[:])
            nc.sync.dma_start(out=out[t], in_=s[:])


def _wrap(ix_tile: np.ndarray) -> np.ndarray:
    """[T] int -> [128, T//16] int16 dma_gather wrap (8x replicated)."""
    t = ix_tile.shape[0]
    a = ix_tile.astype(np.int16).reshape(t // 16, 16).T
    return np.broadcast_to(a[None], (8, 16, t // 16)).reshape(128, t // 16)


def marshal_indices(src, dst, etype, nt, tile_t):
    """Build idx_sd [nt, 128, 2*T/16] (src block then dst block per tile)
    and idx_et [nt, 128, T/16]."""
    isd = np.empty((nt, 128, 2 * tile_t // 16), np.int16)
    iet = np.empty((nt, 128, tile_t // 16), np.int16)
    for t in range(nt):
        sl = slice(t * tile_t, (t + 1) * tile_t)
        isd[t] = _wrap(np.concatenate([src[sl], dst[sl]]))
        iet[t] = _wrap(etype[sl])
    return np.ascontiguousarray(isd), np.ascontiguousarray(iet)


_CACHE = {}
LAST_RESULTS = None  # test.py reads exec_time_ns from here when tracing

VARIANT = os.environ.get("KERNEL_VARIANT", "f16e")
TILE_T = int(os.environ.get("KERNEL_TILE_T", "3200"))


def kernel(h, src, dst, etype, fwd_rel, rev_rel=None):
    global LAST_RESULTS
    from concourse.bass_utils import run_bass_kernel_spmd

    variant, tile_t = VARIANT, TILE_T

    h = np.asarray(h, dtype=np.float32)
    fwd_rel = np.asarray(fwd_rel, dtype=np.float32)
    src = np.asarray(src)
    dst = np.asarray(dst)
    etype = np.asarray(etype)

    nt = CORE_E // tile_t
    key = (CORE_E, tile_t, variant)
    if key not in _CACHE:
        _CACHE[key] = build_program(CORE_E, tile_t, variant)
    nc = _CACHE[key]

    in_maps = []
    for c in range(N_CORES):
        sl = slice(c * CORE_E, (c + 1) * CORE_E)
        isd, iet = marshal_indices(src[sl], dst[sl], etype[sl], nt, tile_t)
        in_maps.append({"h": h, "fwd_rel": fwd_rel, "idx_sd": isd, "idx_et": iet})

    trace = bool(os.environ.get("KERNEL_TRACE"))
    res = run_bass_kernel_spmd(
        nc, in_maps, core_ids=list(range(N_CORES)), trace=trace,
    )
    LAST_RESULTS = res

    outs = []
    for c in range(N_CORES):
        sw = res.results[c]["scores"]  # [nt, 128, tcols]
        outs.append(sw.transpose(0, 2, 1).reshape(CORE_E))
    return np.concatenate(outs).astype(np.float32)


# revision 22
# speedup vs baseline: 8.6643x; 8.6643x over previous
"""DistMult edge scoring on 8 Trainium2 NeuronCores.

score[e] = sum_d h[src[e],d] * fwd_rel[etype[e],d] * h[dst[e],d]

Strategy (edge-parallel): shard the 640k edges across 8 cores (80k each).
Per tile of T edges, each core issues three dma_gather calls (h[src],
h[dst], fwd_rel[etype]) spread across the 4 SWDGE queues — queue-parallel
descriptor generation is what makes the gathers run at fabric bandwidth
(~416 GB/s measured; a single queue is descgen-bound at ~8.7 ns/row).
fp32 rows (512 B) are gathered at line rate; fp16 rows (256 B) would hit
the SDMA read-modify-write penalty and go SLOWER, so everything stays fp32
(bit-exact reductions aside).

Dataflow per tile (edges-on-partitions, [128, T/128, 128] tiles):
  u *= w on DVE (in-place into the gathered u block)
  u *= v on DVE
  score[128, T/128] via ScalarE activation accumulate (per 128-col chunk),
  freeing DVE; DVE-reduce fallback available.

Index tensors are pre-marshalled on host into dma_gather's required wrap:
int16, index i at (partition i%16, column i//16), replicated 8x across the
128 partitions (one copy per GpSimd Q7 core). src/dst are concatenated per
tile into one [128, 2*T/16] tensor. The per-core edge count is padded to a
multiple of T with -1 (skipped by the gather); padded scores are dropped
on the host.

dma_gather calls use single_packet=False: single-packet mode hangs the
device above ~768 indices per call (HW-probed).
"""

import os
import sys

import numpy as np

sys.path.insert(0, "/opt/trn_rl_repo")

import concourse.bass as bass
import concourse.mybir as mybir
from concourse import bacc
from concourse.tile import TileContext

N_NODES = 10000
N_EDGES = 640000
D = 128
NUM_RELS = 500
N_CORES = 8
CORE_E = N_EDGES // N_CORES  # 80000

F32 = mybir.dt.float32
I16 = mybir.dt.int16


def build_program(
    core_e: int,
    tile_t: int,
    repeat: int = 1,
    bufs: int = 3,
    reduce_engine: str = "dve",
    mode: str = "full",  # "full" | "gather_only" | "compute_only"
):
    """Build the per-core Bass program (SPMD: same program, 8 cores).

    repeat>1 re-runs the main loop (same data) for marginal-cost timing.
    """
    assert tile_t % 128 == 0
    nt = -(-core_e // tile_t)  # tiles per core (last may be partial)
    t16 = tile_t // 16
    tcols = tile_t // 128

    nc = bacc.Bacc(num_swdge_queues=4)
    h = nc.declare_dram_parameter("h", [N_NODES, D], F32, isOutput=False)
    rel = nc.declare_dram_parameter("fwd_rel", [NUM_RELS, D], F32, isOutput=False)
    isd = nc.declare_dram_parameter("idx_sd", [nt, 128, 2 * t16], I16, isOutput=False)
    iet = nc.declare_dram_parameter("idx_et", [nt, 128, t16], I16, isOutput=False)
    out = nc.declare_dram_parameter("scores", [nt, 128, tcols], F32, isOutput=True)

    with TileContext(nc) as tc:
        with (
            tc.tile_pool(name="gat", bufs=bufs) as gp,
            tc.tile_pool(name="idx", bufs=bufs) as ip,
            tc.tile_pool(name="res", bufs=bufs) as rp,
        ):
            if mode == "gather_fixed":
                # microbench-mimic: one preloaded idx tile, bare gathers
                ix0 = ip.tile([128, 2 * t16], I16, tag="ix0")
                nc.sync.dma_start(out=ix0[:], in_=isd[0])
                ixe0 = ip.tile([128, t16], I16, tag="ixe0")
                nc.sync.dma_start(out=ixe0[:], in_=iet[0])
                q = 0
                for _ in range(repeat):
                    for t in range(nt):
                        uv = gp.tile([128, 2 * tcols, D], F32, tag="uv")
                        w = gp.tile([128, tcols, D], F32, tag="w")
                        nc.gpsimd.dma_gather(
                            out_ap=uv[:, :tcols, :], in_ap=h[:], idxs_ap=ix0[:, :t16],
                            num_idxs=tile_t, num_idxs_reg=tile_t, elem_size=D,
                            single_packet=False, queue_num=q % 4,
                        )
                        nc.gpsimd.dma_gather(
                            out_ap=uv[:, tcols:, :], in_ap=h[:], idxs_ap=ix0[:, t16:],
                            num_idxs=tile_t, num_idxs_reg=tile_t, elem_size=D,
                            single_packet=False, queue_num=(q + 1) % 4,
                        )
                        nc.gpsimd.dma_gather(
                            out_ap=w[:], in_ap=h[:], idxs_ap=ix0[:, t16:],
                            num_idxs=tile_t, num_idxs_reg=tile_t, elem_size=D,
                            single_packet=False, queue_num=(q + 2) % 4,
                        )
                        q += 3
                s0 = rp.tile([128, tcols], F32, tag="s")
                nc.gpsimd.memset(s0[:], 0.0)
                for t in range(nt):
                    nc.sync.dma_start(out=out[t], in_=s0[:])
                nc.compile()
                return nc
            for _ in range(repeat):
                q = 0
                for t in range(nt):
                    valid = min(tile_t, core_e - t * tile_t)
                    ix_sd = ip.tile([128, 2 * t16], I16, tag="ixsd")
                    ix_w = ip.tile([128, t16], I16, tag="ixw")
                    nc.sync.dma_start(out=ix_sd[:], in_=isd[t])
                    nc.sync.dma_start(out=ix_w[:], in_=iet[t])

                    uv = gp.tile([128, 2 * tcols, D], F32, tag="uv")
                    w = gp.tile([128, tcols, D], F32, tag="w")
                    if valid < tile_t and mode != "compute_only":
                        # tail tile: -1 indices skip the write; zero-fill so
                        # the (discarded) padded scores stay finite
                        nc.gpsimd.memset(uv[:], 0.0)
                        nc.gpsimd.memset(w[:], 0.0)
                    if mode != "compute_only":
                        nc.gpsimd.dma_gather(
                            out_ap=uv[:, :tcols, :], in_ap=h[:], idxs_ap=ix_sd[:, :t16],
                            num_idxs=tile_t, num_idxs_reg=valid, elem_size=D,
                            single_packet=False, queue_num=q % 4,
                        )
                        nc.gpsimd.dma_gather(
                            out_ap=uv[:, tcols:, :], in_ap=h[:], idxs_ap=ix_sd[:, t16:],
                            num_idxs=tile_t, num_idxs_reg=valid, elem_size=D,
                            single_packet=False, queue_num=(q + 1) % 4,
                        )
                        nc.gpsimd.dma_gather(
                            out_ap=w[:], in_ap=rel[:], idxs_ap=ix_w[:],
                            num_idxs=tile_t, num_idxs_reg=valid, elem_size=D,
                            single_packet=False, queue_num=(q + 2) % 4,
                        )
                    q += 3

                    if mode == "gather_only":
                        continue
                    if mode == "compute_only":
                        nc.gpsimd.memset(uv[:], 0.5)
                        nc.gpsimd.memset(w[:], 0.5)

                    u = uv[:, :tcols, :]
                    v = uv[:, tcols:, :]
                    s = rp.tile([128, tcols], F32, tag="s")
                    if reduce_engine == "ttr":
                        # fused: u = u*w on DVE, then per 128-col chunk
                        # (u*v) with a running free-dim accumulate
                        nc.vector.tensor_mul(u, u, w[:])
                        for c in range(tcols):
                            nc.vector.tensor_tensor_reduce(
                                out=uv[:, c, :],
                                in0=uv[:, c, :],
                                in1=uv[:, tcols + c, :],
                                scale=1.0,
                                scalar=0.0,
                                op0=mybir.AluOpType.mult,
                                op1=mybir.AluOpType.add,
                                accum_out=s[:, c : c + 1],
                            )
                        nc.sync.dma_start(out=out[t], in_=s[:])
                        continue
                    nc.vector.tensor_mul(u, u, w[:])
                    nc.vector.tensor_mul(u, u, v)
                    if reduce_engine == "act":
                        for c in range(tcols):
                            nc.scalar.activation(
                                out=uv[:, c, :],
                                in_=uv[:, c, :],
                                func=mybir.ActivationFunctionType.Copy,
                                accum_out=s[:, c : c + 1],
                            )
                    else:
                        nc.vector.reduce_sum(s[:], u, axis=mybir.AxisListType.X)
                    nc.sync.dma_start(out=out[t], in_=s[:])

    nc.compile()
    return nc


def _wrap(ix_tile: np.ndarray) -> np.ndarray:
    """[T] int -> [128, T//16] int16 dma_gather wrap (8x replicated)."""
    t = ix_tile.shape[0]
    a = ix_tile.astype(np.int16).reshape(t // 16, 16).T
    return np.broadcast_to(a[None], (8, 16, t // 16)).reshape(128, t // 16)


def _pad(ix: np.ndarray, n: int) -> np.ndarray:
    if ix.shape[0] == n:
        return ix
    return np.concatenate([ix, np.full(n - ix.shape[0], -1, ix.dtype)])


def marshal_indices(src, dst, etype, nt, tile_t):
    """Build idx_sd [nt, 128, 2*T/16] (src block then dst block per tile)
    and idx_et [nt, 128, T/16], padding the tail tile with -1."""
    core_e = src.shape[0]
    isd = np.empty((nt, 128, 2 * tile_t // 16), np.int16)
    iet = np.empty((nt, 128, tile_t // 16), np.int16)
    for t in range(nt):
        sl = slice(t * tile_t, min((t + 1) * tile_t, core_e))
        s_t = _pad(src[sl], tile_t)
        d_t = _pad(dst[sl], tile_t)
        isd[t, :, : tile_t // 16] = _wrap(s_t)
        isd[t, :, tile_t // 16 :] = _wrap(d_t)
        iet[t] = _wrap(_pad(etype[sl], tile_t))
    return np.ascontiguousarray(isd), np.ascontiguousarray(iet)


_CACHE = {}
LAST_RESULTS = None  # test.py reads exec_time_ns from here when tracing

TILE_T = int(os.environ.get("KERNEL_TILE_T", "2048"))
BUFS = int(os.environ.get("KERNEL_BUFS", "8"))
REDUCE = os.environ.get("KERNEL_REDUCE", "dve")


def kernel(h, src, dst, etype, fwd_rel, rev_rel=None):
    global LAST_RESULTS
    from concourse.bass_utils import run_bass_kernel_spmd

    tile_t = TILE_T

    h = np.asarray(h, dtype=np.float32)
    fwd_rel = np.asarray(fwd_rel, dtype=np.float32)
    src = np.asarray(src)
    dst = np.asarray(dst)
    etype = np.asarray(etype)

    nt = -(-CORE_E // tile_t)
    key = (CORE_E, tile_t, BUFS, REDUCE)
    if key not in _CACHE:
        _CACHE[key] = build_program(CORE_E, tile_t, bufs=BUFS, reduce_engine=REDUCE)
    nc = _CACHE[key]

    in_maps = []
    for c in range(N_CORES):
        sl = slice(c * CORE_E, (c + 1) * CORE_E)
        isd, iet = marshal_indices(src[sl], dst[sl], etype[sl], nt, tile_t)
        in_maps.append({"h": h, "fwd_rel": fwd_rel, "idx_sd": isd, "idx_et": iet})

    trace = bool(os.environ.get("KERNEL_TRACE"))
    res = run_bass_kernel_spmd(
        nc, in_maps, core_ids=list(range(N_CORES)), trace=trace,
    )
    LAST_RESULTS = res

    outs = []
    for c in range(N_CORES):
        sw = res.results[c]["scores"]  # [nt, 128, tcols]
        outs.append(sw.transpose(0, 2, 1).reshape(nt * tile_t)[:CORE_E])
    return np.concatenate(outs).astype(np.float32)
